# revision 1
# baseline (speedup 1.0000x reference)
"""BinaryConnect 3x3 SAME conv (NHWC, 32x112x112x128 -> 32x112x112x256) on 8 trn2 cores.

Strategy (data-parallel, 4 images per core, duty-tuned fp8-DoubleRow / fp16):
  - Host: binarize kernel to +/-1 (exact). Quantize x to e4m3 (x8, rows padded
    to 128B for the DoubleRow k-tile step constraint) AND fp16 (x16, rows of
    114), both channel-major with a 1-px zero halo.
  - Device: each output tile (4 rows x 112 cols, one cout half) accumulates
    its 9 conv taps in PSUM. Groups alternate composition to keep the chip
    under its power-throttle threshold (fp8 DoubleRow draws ~2x array power;
    >~22% DR duty trips a chip-wide 2.4->2.0 GHz P-state):
      * even groups: 2 DR pair slots (taps (0,0)+(1,0), (0,1)+(1,1)) + 5 fp16
      * odd groups:  1 DR pair slot (taps (0,0)+(1,0)) + 7 fp16
    DR pairs use an overlapping 4D AP whose k-tile dim steps one image row.
    The DR slots of a block of 2-4 groups are issued back-to-back before the
    block's fp16 slots: the PE pays its ~16ns DR->fp16 transition once per
    block instead of once per group.
  - Output fp16, un-transposed + cast to fp32 on host.
"""

import os

import numpy as np
import ml_dtypes

import concourse.bass as bass
import concourse.mybir as mybir
import concourse.tile as tile
from concourse import bacc
from concourse.bass_utils import run_bass_kernel_spmd

N_CORES = 8
NPC = 4            # images per core
H = 112
WP8 = 128          # fp8 padded row width (16B-aligned DoubleRow k-tile step)
WP6 = 114          # fp16 padded row width
HP = 115           # 1 top pad + 112 rows + 1 bottom pad + 1 zero tail row
CI = 128
CO = 256
TROWS = 4          # output rows per matmul tile
S = TROWS * H      # 448 matmul free dim (<=512 fp32 PSUM bank)
BROWS = 28         # output rows per input band
NB = H // BROWS    # 4 bands per image
BIN = BROWS + 3    # input rows per band incl. halo
TSB = BROWS // TROWS  # 7 tiles per band

# tap ids 0..8 in (dh,dw) raster order; fp16 weight tile indexed by tap id
TAPS = [(dh, dw) for dh in range(3) for dw in range(3)]
TID = {t: i for i, t in enumerate(TAPS)}
PAIR_SLOTS = [((0, 0), (1, 0)), ((0, 1), (1, 1))]  # DR slot s pairs these taps

F8 = ml_dtypes.float8_e4m3
DR = mybir.MatmulPerfMode.DoubleRow

_nc_cache = None
LAST_RESULT = None


def _npairs(gidx):
    return 2 if gidx % 2 == 0 else 1


def _build():
    nc = bacc.Bacc(
        "TRN2",
        target_bir_lowering=False,
        debug=False,
        num_devices=N_CORES,
    )
    x8_d = nc.dram_tensor(
        "x8", [CI, NPC, HP, WP8], mybir.dt.float8e4, kind="ExternalInput"
    )
    x6_d = nc.dram_tensor(
        "x16", [CI, NPC, HP, WP6], mybir.dt.float16, kind="ExternalInput"
    )
    w8_d = nc.dram_tensor(
        "w8", [CI, 2, 2, 2, 128], mybir.dt.float8e4, kind="ExternalInput"
    )
    w6_d = nc.dram_tensor(
        "w16", [CI, 2, 9 * 128], mybir.dt.float16, kind="ExternalInput"
    )
    o_d = nc.dram_tensor(
        "out_cm", [CO, NPC, H * H], mybir.dt.float16, kind="ExternalOutput"
    )
    with tile.TileContext(nc) as tc:
        with (
            tc.tile_pool(name="xpool", bufs=1) as xpool,
            tc.tile_pool(name="wpool", bufs=1) as wpool,
            tc.tile_pool(name="psum", bufs=8, space=bass.MemorySpace.PSUM) as psum,
            tc.tile_pool(name="opool", bufs=10) as opool,
        ):
            # Warmup operand with no DMA dependency (memset) so PE warmup can
            # start right after the framework preamble.
            wta = wpool.tile([CI, S], mybir.dt.float16, tag="wta", name="wta")
            nc.gpsimd.memset(wta[:], 0.0)
            w8t = wpool.tile([CI, 2, 2, 2, 128], mybir.dt.float8e4, tag="w8", name="w8")
            nc.sync.dma_start(w8t[:], w8_d[:])
            # Small first chunks of image 0 (rows 0-11) on the ACT ring so the
            # first matmul groups gate on small DMAs.
            xa8 = xpool.tile([CI, 12, WP8], mybir.dt.float8e4, tag="xa8", name="xa8")
            nc.scalar.dma_start(xa8[:], x8_d[:, 0, 0:12, :])
            xa6 = xpool.tile([CI, 12, WP6], mybir.dt.float16, tag="xa6", name="xa6")
            nc.scalar.dma_start(xa6[:], x6_d[:, 0, 0:12, :])
            w6t = wpool.tile([CI, 2, 9 * 128], mybir.dt.float16, tag="w6", name="w6")
            nc.sync.dma_start(w6t[:], w6_d[:])
            # PE warmup: throwaway matmuls to reach HAM K=8/8 before the real
            # stream begins.
            wu = psum.tile([128, S], mybir.dt.float32, name="ps")
            for _ in range(9):
                nc.tensor.matmul(
                    wu[:], wta[:, 0:128], wta[:, 0:S], start=True, stop=True
                )
            xs8, xs6 = {}, {}
            for b in range(NB):
                r0b = b * BROWS
                xt = xpool.tile([CI, BIN, WP8], mybir.dt.float8e4,
                                tag=f"a0_{b}", name=f"a0_{b}")
                xt6 = xpool.tile([CI, BIN, WP6], mybir.dt.float16,
                                 tag=f"b0_{b}", name=f"b0_{b}")
                if b == 0:
                    # band 0 gates early compute: split its DMAs into chunks so
                    # they spread across queues and arrive sooner.
                    for (ra, rb) in [(0, 16), (16, BIN)]:
                        nc.sync.dma_start(
                            xt[:, ra:rb, :], x8_d[:, 0, r0b + ra : r0b + rb, :]
                        )
                        nc.sync.dma_start(
                            xt6[:, ra:rb, :], x6_d[:, 0, r0b + ra : r0b + rb, :]
                        )
                else:
                    nc.sync.dma_start(xt[:], x8_d[:, 0, r0b : r0b + BIN, :])
                    nc.sync.dma_start(xt6[:], x6_d[:, 0, r0b : r0b + BIN, :])
                xs8[0, b] = xt
                xs6[0, b] = xt6
            for n in range(1, NPC):
                xt = xpool.tile([CI, HP, WP8], mybir.dt.float8e4,
                                tag=f"ai{n}", name=f"ai{n}")
                nc.sync.dma_start(xt[:], x8_d[:, n, :, :])
                for b in range(NB):
                    xs8[n, b] = xt
                xt = xpool.tile([CI, HP, WP6], mybir.dt.float16,
                                tag=f"bi{n}", name=f"bi{n}")
                nc.sync.dma_start(xt[:], x6_d[:, n, :, :])
                for b in range(NB):
                    xs6[n, b] = xt
            pairs = [(0, 1), (2, 3), (4, 5), (6,)]
            gctr = [0]

            def emit_block(n, b, worklist, ots):
                """worklist: [(j, st, half, ot)] — DR slots of every group
                first, then fp16 slots + cast per group."""
                groups = []
                for (j, st, half, ot) in worklist:
                    gidx = gctr[0]
                    gctr[0] += 1
                    npair = _npairs(gidx)
                    ps = psum.tile([128, S], mybir.dt.float32, name="ps")
                    groups.append((j, st, half, ot, npair, ps))
                early = n == 0 and b == 0 and worklist[0][1] <= 1

                def srcs(st):
                    if n == 0:
                        r0 = st * TROWS
                        if early:
                            return r0, xa8, xa6
                        return r0, xs8[n, b], xs6[n, b]
                    return b * BROWS + st * TROWS, xs8[n, b], xs6[n, b]

                for (j, st, half, ot, npair, ps) in groups:
                    r0, s8, s6 = srcs(st)
                    for si in range(npair):
                        dw = si  # pair slot si covers taps (0,si),(1,si)
                        nat = s8[:, r0 : r0 + TROWS, dw : dw + H]
                        pstep = nat.ap[0][0]
                        rhs = bass.AP(
                            nat.tensor, r0 * WP8 + dw,
                            [[pstep, CI], [WP8, 2], [WP8, TROWS], [1, H]],
                        )
                        nc.tensor.matmul(
                            ps[:], w8t[:, half, si, :, :], rhs,
                            start=(si == 0), stop=False, perf_mode=DR,
                        )
                for (j, st, half, ot, npair, ps) in groups:
                    r0, s8, s6 = srcs(st)
                    ptaps = {t for sl in PAIR_SLOTS[:npair] for t in sl}
                    f16taps = [t for t in TAPS if t not in ptaps]
                    for i, (dh, dw) in enumerate(f16taps):
                        t = TID[(dh, dw)]
                        rhs = s6[:, r0 + dh : r0 + dh + TROWS, dw : dw + H]
                        nc.tensor.matmul(
                            ps[:], w6t[:, half, t * 128 : t * 128 + 128], rhs,
                            start=False, stop=(i == len(f16taps) - 1),
                        )
                    nc.vector.tensor_copy(ot[:, j * S : (j + 1) * S], ps[:])

            def emit_dma(n, b, half, sts, ot):
                width = len(sts) * S
                o0 = (b * BROWS + sts[0] * TROWS) * H
                nc.scalar.dma_start(
                    o_d[half * 128 : half * 128 + 128, n, o0 : o0 + width],
                    ot[:, 0:width],
                )

            for n in range(NPC):
                for b in range(NB):
                    for sts in pairs:
                        if (n, b, sts) == (0, 0, (0, 1)):
                            ots = [
                                opool.tile([128, 2 * S], mybir.dt.float16, name="ot")
                                for _ in range(2)
                            ]
                            emit_block(n, b, [
                                (0, sts[0], 0, ots[0]),
                                (0, sts[0], 1, ots[1]),
                                (1, sts[1], 0, ots[0]),
                                (1, sts[1], 1, ots[1]),
                            ], ots)
                            for half in range(2):
                                emit_dma(n, b, half, sts, ots[half])
                        else:
                            for half in range(2):
                                ot = opool.tile(
                                    [128, 2 * S], mybir.dt.float16, name="ot"
                                )
                                emit_block(n, b, [
                                    (j, st, half, ot) for j, st in enumerate(sts)
                                ], [ot])
                                emit_dma(n, b, half, sts, ot)
    nc.compile()
    return nc


def _get_nc():
    global _nc_cache
    if _nc_cache is None:
        _nc_cache = _build()
    return _nc_cache


def kernel(x, kernel):
    global LAST_RESULT
    x = np.asarray(x)
    k = np.asarray(kernel)

    wb = np.where(k >= 0, np.float32(1), np.float32(-1))  # [3,3,128,256]
    w8 = np.zeros((CI, 2, 2, 2, 128), np.float32)
    for half in range(2):
        co = slice(half * 128, half * 128 + 128)
        for si, ((dhA, dwA), (dhB, dwB)) in enumerate(PAIR_SLOTS):
            w8[:, half, si, 0, :] = wb[dhA, dwA, :, co]
            w8[:, half, si, 1, :] = wb[dhB, dwB, :, co]
    w8 = np.ascontiguousarray(w8.astype(F8))
    w16 = np.zeros((CI, 2, 9 * 128), np.float16)
    for half in range(2):
        co = slice(half * 128, half * 128 + 128)
        for t, (dh, dw) in enumerate(TAPS):
            w16[:, half, t * 128 : t * 128 + 128] = wb[dh, dw, :, co]

    x8 = x.astype(F8)
    x16 = x.astype(np.float16)

    in_maps = []
    for c in range(N_CORES):
        sl = slice(c * NPC, (c + 1) * NPC)
        xp8 = np.zeros((CI, NPC, HP, WP8), F8)
        xp8[:, :, 1:113, 1:113] = x8[sl].transpose(3, 0, 1, 2)
        xp6 = np.zeros((CI, NPC, HP, WP6), np.float16)
        xp6[:, :, 1:113, 1:113] = x16[sl].transpose(3, 0, 1, 2)
        in_maps.append({"x8": xp8, "x16": xp6, "w8": w8, "w16": w16})

    nc = _get_nc()
    trace = os.environ.get("BCONV_TRACE", "0") == "1"
    kwargs = {}
    if trace and os.environ.get("BCONV_TRACE_CORES", "") == "all":
        kwargs["trace_cores"] = list(range(N_CORES))
    res = run_bass_kernel_spmd(
        nc, in_maps, core_ids=list(range(N_CORES)), trace=trace, **kwargs
    )
    LAST_RESULT = res

    out = np.empty((32, H, H, CO), np.float32)
    for c in range(N_CORES):
        o = res.results[c]["out_cm"].reshape(CO, NPC, H, H).astype(np.float32)
        out[c * NPC : (c + 1) * NPC] = o.transpose(1, 2, 3, 0)
    return out



# revision 7
# speedup vs baseline: 1.0651x; 1.0651x over previous
"""BinaryConnect 3x3 SAME conv (NHWC, 32x112x112x128 -> 32x112x112x256) on 8 trn2 cores.

Strategy: data-parallel (4 images/core) + 1D Winograd F(2,3) along W.
  - Host: binarize kernel (exact), 1D-Winograd-transform weights
    (coeffs in {+-1, +-1/2, +-3/2}: exact in fp16/fp8) and activations
    (4 xi planes per 2 output cols, computed in fp32, cast fp16; the two
    "edge" planes xi0=d0-d2, xi3=d1-d3 also cast to e4m3 fp8).
  - Device: m[xi] = sum_dh w~[dh,xi] . x~[row+dh, xi] accumulated in PSUM
    (3 dh matmuls per xi, 4 xi per output tile). For xi0/xi3 the dh0+dh1
    matmuls are fused into one fp8 DoubleRow matmul (k-tile steps one
    image row) -> 10 matmul slots per tile instead of direct conv's 9,
    but each covers HALF the pixels (56 col-tiles vs 112 cols): 10/18 of
    direct fp16 work. DR duty 2/10 = 20% stays under the ~22% chip
    power-throttle threshold. fp8 on the edge planes only (their error
    enters one output phase, not two): rel err ~1.76e-2 < 2e-2.
  - Vector engine reconstructs outputs from PSUM via 2 fused
    scalar_tensor_tensor ops per phase: y_even = m0+m1+m2,
    y_odd = m1-m2-m3; writes fp16 even/odd planes, host interleaves.
"""

import os

import numpy as np
import ml_dtypes

import concourse.bass as bass
import concourse.mybir as mybir
import concourse.tile as tile
from concourse import bacc
from concourse.bass_utils import run_bass_kernel_spmd

N_CORES = 8
NPC = 4            # images per core
H = 112
CT = 56            # col tiles (2 out cols each)
HP = 114           # 1 top pad + 112 rows + 1 bottom pad
W6 = 4 * CT        # fp16 x~ row width (4 xi planes)
W8 = 2 * CT        # fp8 x~ row width (xi0, xi3)
CI = 128
CO = 256
TROWS = 7          # output rows per matmul tile
S = TROWS * CT     # 392 matmul free dim (<=512 fp32 PSUM bank)
BROWS = 28         # output rows per band
NB = H // BROWS    # 4 bands per image
BIN = BROWS + 2    # x~ rows per band (1 halo row each side)
TSB = BROWS // TROWS  # 4 tiles per band
NBUF = 6           # x~ band ring depth

F8 = ml_dtypes.float8_e4m3
DR = mybir.MatmulPerfMode.DoubleRow
ADD = mybir.AluOpType.add
MULT = mybir.AluOpType.mult

_nc_cache = None
LAST_RESULT = None


def _build():
    nc = bacc.Bacc(
        "TRN2",
        target_bir_lowering=False,
        debug=False,
        num_devices=N_CORES,
    )
    x8_d = nc.dram_tensor(
        "x8", [CI, NPC, HP, W8], mybir.dt.float8e4, kind="ExternalInput"
    )
    x6_d = nc.dram_tensor(
        "x16", [CI, NPC, HP, W6], mybir.dt.float16, kind="ExternalInput"
    )
    w8_d = nc.dram_tensor(
        "w8", [CI, 2, 2, 2, 128], mybir.dt.float8e4, kind="ExternalInput"
    )
    w6_d = nc.dram_tensor(
        "w16", [CI, 2, 12 * 128], mybir.dt.float16, kind="ExternalInput"
    )
    # out: [co, n, tile(16), phase(2), S]
    o_d = nc.dram_tensor(
        "out_cm", [CO, NPC, NB * TSB * 2 * S], mybir.dt.float16,
        kind="ExternalOutput"
    )
    with tile.TileContext(nc) as tc:
        with (
            tc.tile_pool(name="x8pool", bufs=NBUF) as x8pool,
            tc.tile_pool(name="x16pool", bufs=NBUF) as x16pool,
            tc.tile_pool(name="wpool", bufs=1) as wpool,
            tc.tile_pool(name="tpool", bufs=6) as tpool,
            tc.tile_pool(name="psum", bufs=8, space=bass.MemorySpace.PSUM) as psum,
            tc.tile_pool(name="opool", bufs=8) as opool,
        ):
            # Warmup operand with no DMA dependency (memset) so PE warmup can
            # start right after the framework preamble.
            wta = wpool.tile([CI, S], mybir.dt.float16, tag="wta", name="wta")
            nc.gpsimd.memset(wta[:], 0.0)
            w8t = wpool.tile([CI, 2, 2, 2, 128], mybir.dt.float8e4, tag="w8", name="w8")
            nc.sync.dma_start(w8t[:], w8_d[:])
            w6t = wpool.tile([CI, 2, 12 * 128], mybir.dt.float16, tag="w6", name="w6")
            nc.sync.dma_start(w6t[:], w6_d[:])
            # PE warmup: throwaway matmuls to reach HAM K=8/8 before the real
            # stream begins.
            wu = psum.tile([128, S], mybir.dt.float32, name="ps")
            for _ in range(9):
                nc.tensor.matmul(
                    wu[:], wta[:, 0:128], wta[:, 0:S], start=True, stop=True
                )
            # x~ band ring: band index k = (n*NB + b), buffer k % NBUF.
            xs8, xs6 = {}, {}

            def load_band(n, b):
                k = n * NB + b
                r0 = b * BROWS
                xt8 = x8pool.tile([CI, BIN, W8], mybir.dt.float8e4, name="xb8")
                xt6 = x16pool.tile([CI, BIN, W6], mybir.dt.float16, name="xb6")
                if k == 0:
                    # first band gates early compute: split DMAs into
                    # per-tile chunks so they arrive sooner, spread queues.
                    bounds = [0, 9, 16, 23, BIN]
                    for i in range(4):
                        ra, rb = bounds[i], bounds[i + 1]
                        q = nc.scalar if i == 0 else nc.sync
                        q.dma_start(xt8[:, ra:rb, :], x8_d[:, n, r0 + ra: r0 + rb, :])
                        q.dma_start(xt6[:, ra:rb, :], x6_d[:, n, r0 + ra: r0 + rb, :])
                else:
                    nc.sync.dma_start(xt8[:], x8_d[:, n, r0: r0 + BIN, :])
                    nc.sync.dma_start(xt6[:], x6_d[:, n, r0: r0 + BIN, :])
                xs8[n, b] = xt8
                xs6[n, b] = xt6

            # prefetch the first NBUF bands up front
            order = [(n, b) for n in range(NPC) for b in range(NB)]
            for (n, b) in order[:NBUF]:
                load_band(n, b)

            def emit_group(n, b, t, half):
                s8 = xs8[n, b]
                s6 = xs6[n, b]
                r0 = t * TROWS  # band-local x~ row base
                ps = [psum.tile([128, S], mybir.dt.float32, name="ps")
                      for _ in range(4)]
                # DR slots: xi0 (si=0), xi3 (si=1); k-tile = (dh0, dh1).
                for si in range(2):
                    nat = s8[:, r0: r0 + TROWS, si * CT: si * CT + CT]
                    pstep = nat.ap[0][0]
                    rhs = bass.AP(
                        nat.tensor, r0 * W8 + si * CT,
                        [[pstep, CI], [W8, 2], [W8, TROWS], [1, CT]],
                    )
                    xi = 0 if si == 0 else 3
                    nc.tensor.matmul(
                        ps[xi][:], w8t[:, half, si, :, :], rhs,
                        start=True, stop=False, perf_mode=DR,
                    )
                # fp16 slots, ordered so m0/m1 complete first.
                f16 = [(0, 2, False, True), (1, 0, True, False),
                       (1, 1, False, False), (1, 2, False, True),
                       (2, 0, True, False), (2, 1, False, False),
                       (2, 2, False, True), (3, 2, False, True)]
                for (xi, dh, st_, sp_) in f16:
                    idx = xi * 3 + dh
                    rhs = s6[:, r0 + dh: r0 + dh + TROWS,
                             xi * CT: xi * CT + CT]
                    nc.tensor.matmul(
                        ps[xi][:], w6t[:, half, idx * 128: idx * 128 + 128],
                        rhs, start=st_, stop=sp_,
                    )
                # inverse transform: even = m0+m1+m2, odd = m1-(m2+m3).
                # Vector ops may read only ONE PSUM operand: ACT engine
                # stages m0/m3 into SBUF, vector fuses with the PSUM reads.
                ot = opool.tile([128, 2, S], mybir.dt.float16, name="ot")
                tm = tpool.tile([128, 4, S], mybir.dt.float32, name="tm")
                nc.scalar.copy(tm[:, 2, :], ps[0][:])
                nc.vector.scalar_tensor_tensor(
                    tm[:, 0, :], tm[:, 2, :], 1.0, ps[1][:], MULT, ADD)
                nc.vector.scalar_tensor_tensor(
                    ot[:, 0, :], tm[:, 0, :], 1.0, ps[2][:], MULT, ADD)
                nc.scalar.copy(tm[:, 3, :], ps[3][:])
                nc.vector.scalar_tensor_tensor(
                    tm[:, 1, :], tm[:, 3, :], 1.0, ps[2][:], MULT, ADD)
                nc.vector.scalar_tensor_tensor(
                    ot[:, 1, :], tm[:, 1, :], -1.0, ps[1][:], MULT, ADD)
                # out DMA: tile index T = b*TSB + t
                T = b * TSB + t
                nc.scalar.dma_start(
                    o_d[half * 128: half * 128 + 128, n,
                        T * 2 * S: (T + 1) * 2 * S],
                    ot[:],
                )

            nextband = NBUF
            for ki, (n, b) in enumerate(order):
                for t in range(TSB):
                    for half in range(2):
                        emit_group(n, b, t, half)
                if nextband < len(order):
                    nn, nb_ = order[nextband]
                    load_band(nn, nb_)
                    nextband += 1
    nc.compile()
    return nc


def _get_nc():
    global _nc_cache
    if _nc_cache is None:
        _nc_cache = _build()
    return _nc_cache


def kernel(x, kernel):
    global LAST_RESULT
    x = np.asarray(x)
    k = np.asarray(kernel)

    wb = np.where(k >= 0, np.float32(1), np.float32(-1))  # [3,3,128,256]
    # 1D winograd weight transform along W: per dh, 4 xi planes.
    # gt[dh][xi] : [128ci, 256co]
    gt = [[wb[dh, 0],
           (wb[dh, 0] + wb[dh, 1] + wb[dh, 2]) * 0.5,
           (wb[dh, 0] - wb[dh, 1] + wb[dh, 2]) * 0.5,
           wb[dh, 2]] for dh in range(3)]
    # fp8 DR weights: slot si in {0: xi0, 1: xi3}, ktile in {dh0, dh1}
    w8 = np.zeros((CI, 2, 2, 2, 128), np.float32)
    for half in range(2):
        co = slice(half * 128, half * 128 + 128)
        for si, xi in enumerate((0, 3)):
            w8[:, half, si, 0, :] = gt[0][xi][:, co]
            w8[:, half, si, 1, :] = gt[1][xi][:, co]
    w8 = np.ascontiguousarray(w8.astype(F8))
    # fp16 weights: idx = xi*3 + dh
    w16 = np.zeros((CI, 2, 12 * 128), np.float16)
    for half in range(2):
        co = slice(half * 128, half * 128 + 128)
        for xi in range(4):
            for dh in range(3):
                idx = xi * 3 + dh
                w16[:, half, idx * 128: idx * 128 + 128] = gt[dh][xi][:, co]

    # activation transform (fp32), pad H and W by 1.
    xp = np.pad(x, ((0, 0), (1, 1), (1, 1), (0, 0)))  # [32,114,114,128]
    c_idx = np.arange(CT) * 2
    D = [xp[:, :, c_idx + j, :] for j in range(4)]  # [32,114,56,128]
    XT = [D[0] - D[2], D[1] + D[2], D[2] - D[1], D[1] - D[3]]
    # XT[xi]: [32,114,56,128] -> want [128, n, 114, xi, 56]
    xt6 = np.stack(XT, axis=2)          # [32, 114, 4, 56, 128] fp32
    xt8 = np.stack((XT[0], XT[3]), axis=2)  # [32, 114, 2, 56, 128]
    xt6 = np.ascontiguousarray(
        xt6.transpose(4, 0, 1, 2, 3).astype(np.float16))  # [128,32,114,4,56]
    xt8 = np.ascontiguousarray(
        xt8.transpose(4, 0, 1, 2, 3).astype(F8))          # [128,32,114,2,56]

    in_maps = []
    for c in range(N_CORES):
        sl = slice(c * NPC, (c + 1) * NPC)
        in_maps.append({
            "x8": xt8[:, sl].reshape(CI, NPC, HP, W8),
            "x16": xt6[:, sl].reshape(CI, NPC, HP, W6),
            "w8": w8, "w16": w16,
        })

    nc = _get_nc()
    trace = os.environ.get("BCONV_TRACE", "0") == "1"
    kwargs = {}
    if trace and os.environ.get("BCONV_TRACE_CORES", "") == "all":
        kwargs["trace_cores"] = list(range(N_CORES))
    res = run_bass_kernel_spmd(
        nc, in_maps, core_ids=list(range(N_CORES)), trace=trace, **kwargs
    )
    LAST_RESULT = res

    out = np.empty((32, H, H, CO), np.float32)
    for c in range(N_CORES):
        o = res.results[c]["out_cm"].reshape(CO, NPC, NB * TSB, 2, TROWS, CT)
        # row = 7*T + r ; col = 2*c + phase
        y = o.transpose(1, 2, 4, 5, 3, 0).reshape(NPC, H, H, CO)
        out[c * NPC: (c + 1) * NPC] = y.astype(np.float32)
    return out


# revision 10
# speedup vs baseline: 1.1627x; 1.0917x over previous
"""BinaryConnect 3x3 SAME conv (NHWC, 32x112x112x128 -> 32x112x112x256) on 8 trn2 cores.

Strategy: data-parallel (4 images/core) + 1D Winograd F(2,3) along W.
  - Host: binarize kernel (exact), 1D-Winograd-transform weights
    (coeffs in {+-1, +-1/2, +-3/2}: exact in fp16/fp8) and activations
    (4 xi planes per 2 output cols, computed in fp32, cast fp16; the two
    "edge" planes xi0=d0-d2, xi3=d1-d3 also cast to e4m3 fp8).
  - Device: m[xi] = sum_dh w~[dh,xi] . x~[row+dh, xi] accumulated in PSUM
    (3 dh matmuls per xi, 4 xi per output tile). For xi0/xi3 the dh0+dh1
    matmuls are fused into one fp8 DoubleRow matmul (k-tile steps one
    image row) -> 10 matmul slots per tile instead of direct conv's 9,
    but each covers HALF the pixels (56 col-tiles vs 112 cols): 10/18 of
    direct fp16 work. DR duty 2/10 = 20% stays under the ~22% chip
    power-throttle threshold. fp8 on the edge planes only (their error
    enters one output phase, not two): rel err ~1.76e-2 < 2e-2.
  - Vector engine reconstructs outputs from PSUM via 2 fused
    scalar_tensor_tensor ops per phase: y_even = m0+m1+m2,
    y_odd = m1-m2-m3; writes fp16 even/odd planes, host interleaves.
"""

import os

import numpy as np
import ml_dtypes

import concourse.bass as bass
import concourse.mybir as mybir
import concourse.tile as tile
from concourse import bacc
from concourse.bass_utils import run_bass_kernel_spmd

N_CORES = 8
NPC = 4            # images per core
H = 112
CT = 56            # col tiles (2 out cols each)
HP = 114           # 1 top pad + 112 rows + 1 bottom pad
W6 = 4 * CT        # fp16 x~ row width (4 xi planes)
W8 = 2 * CT        # fp8 x~ row width (xi0, xi3)
CI = 128
CO = 256
TROWS = 8          # output rows per matmul tile
S = TROWS * CT     # 448 matmul free dim (<=512 fp32 PSUM bank)
BROWS = 56         # output rows per band
NB = H // BROWS    # 2 bands per image
BIN = BROWS + 2    # x~ rows per band (1 halo row each side)
TSB = BROWS // TROWS  # 7 tiles per band
NBUF = 4           # x~ band ring depth

F8 = ml_dtypes.float8_e4m3
DR = mybir.MatmulPerfMode.DoubleRow
ADD = mybir.AluOpType.add
MULT = mybir.AluOpType.mult

_nc_cache = None
LAST_RESULT = None


def _build():
    nc = bacc.Bacc(
        "TRN2",
        target_bir_lowering=False,
        debug=False,
        num_devices=N_CORES,
    )
    x8_d = nc.dram_tensor(
        "x8", [CI, NPC, HP, W8], mybir.dt.float8e4, kind="ExternalInput"
    )
    x6_d = nc.dram_tensor(
        "x16", [CI, NPC, HP, W6], mybir.dt.float16, kind="ExternalInput"
    )
    w8_d = nc.dram_tensor(
        "w8", [CI, 2, 2, 2, 128], mybir.dt.float8e4, kind="ExternalInput"
    )
    w6_d = nc.dram_tensor(
        "w16", [CI, 2, 12 * 128], mybir.dt.float16, kind="ExternalInput"
    )
    # out: [co, n, tile(16), phase(2), S]
    o_d = nc.dram_tensor(
        "out_cm", [CO, NPC, NB * TSB * 2 * S], mybir.dt.float16,
        kind="ExternalOutput"
    )
    with tile.TileContext(nc) as tc:
        with (
            tc.tile_pool(name="x8pool", bufs=NBUF) as x8pool,
            tc.tile_pool(name="x16pool", bufs=NBUF) as x16pool,
            tc.tile_pool(name="wpool", bufs=1) as wpool,
            tc.tile_pool(name="tpool", bufs=4) as tpool,
            tc.tile_pool(name="psum", bufs=8, space=bass.MemorySpace.PSUM) as psum,
            tc.tile_pool(name="opool", bufs=8) as opool,
        ):
            # Warmup operand with no DMA dependency (memset) so PE warmup can
            # start right after the framework preamble.
            wta = wpool.tile([CI, S], mybir.dt.float16, tag="wta", name="wta")
            nc.gpsimd.memset(wta[:], 0.0)
            w8t = wpool.tile([CI, 2, 2, 2, 128], mybir.dt.float8e4, tag="w8", name="w8")
            nc.sync.dma_start(w8t[:], w8_d[:])
            w6t = wpool.tile([CI, 2, 12 * 128], mybir.dt.float16, tag="w6", name="w6")
            nc.sync.dma_start(w6t[:], w6_d[:])
            # PE warmup: throwaway matmuls to reach HAM K=8/8 before the real
            # stream begins.
            wu = psum.tile([128, S], mybir.dt.float32, name="ps")
            for _ in range(9):
                nc.tensor.matmul(
                    wu[:], wta[:, 0:128], wta[:, 0:S], start=True, stop=True
                )
            # x~ band ring: band index k = (n*NB + b), buffer k % NBUF.
            xs8, xs6 = {}, {}

            def load_band(n, b):
                k = n * NB + b
                r0 = b * BROWS
                xt8 = x8pool.tile([CI, BIN, W8], mybir.dt.float8e4, name="xb8")
                xt6 = x16pool.tile([CI, BIN, W6], mybir.dt.float16, name="xb6")
                if k == 0:
                    # first band gates early compute: split DMAs into
                    # per-tile chunks so they arrive sooner, spread queues.
                    bounds = [0, 10, 18, 26, 34, 42, 50, BIN]
                    for i in range(7):
                        ra, rb = bounds[i], bounds[i + 1]
                        q = nc.scalar if i == 0 else nc.sync
                        q.dma_start(xt8[:, ra:rb, :], x8_d[:, n, r0 + ra: r0 + rb, :])
                        q.dma_start(xt6[:, ra:rb, :], x6_d[:, n, r0 + ra: r0 + rb, :])
                else:
                    nc.sync.dma_start(xt8[:], x8_d[:, n, r0: r0 + BIN, :])
                    nc.sync.dma_start(xt6[:], x6_d[:, n, r0: r0 + BIN, :])
                xs8[n, b] = xt8
                xs6[n, b] = xt6

            # prefetch the first NBUF bands up front
            order = [(n, b) for n in range(NPC) for b in range(NB)]
            for (n, b) in order[:NBUF]:
                load_band(n, b)

            def emit_group(n, b, t, half):
                s8 = xs8[n, b]
                s6 = xs6[n, b]
                r0 = t * TROWS  # band-local x~ row base
                ps = [psum.tile([128, S], mybir.dt.float32, name="ps")
                      for _ in range(4)]
                # DR slots: xi0 (si=0), xi3 (si=1); k-tile = (dh0, dh1).
                for si in range(2):
                    nat = s8[:, r0: r0 + TROWS, si * CT: si * CT + CT]
                    pstep = nat.ap[0][0]
                    rhs = bass.AP(
                        nat.tensor, r0 * W8 + si * CT,
                        [[pstep, CI], [W8, 2], [W8, TROWS], [1, CT]],
                    )
                    xi = 0 if si == 0 else 3
                    nc.tensor.matmul(
                        ps[xi][:], w8t[:, half, si, :, :], rhs,
                        start=True, stop=False, perf_mode=DR,
                    )
                # fp16 slots, ordered so m0/m3/m2 complete early (the
                # PSUM->SBUF staging chains hang off m2 and m1).
                f16 = [(0, 2, False, True), (3, 2, False, True),
                       (2, 0, True, False), (2, 1, False, False),
                       (2, 2, False, True), (1, 0, True, False),
                       (1, 1, False, False), (1, 2, False, True)]
                for (xi, dh, st_, sp_) in f16:
                    idx = xi * 3 + dh
                    rhs = s6[:, r0 + dh: r0 + dh + TROWS,
                             xi * CT: xi * CT + CT]
                    nc.tensor.matmul(
                        ps[xi][:], w6t[:, half, idx * 128: idx * 128 + 128],
                        rhs, start=st_, stop=sp_,
                    )
                # inverse transform: even = m0+m1+m2, odd = m1-(m2+m3).
                # Engine constraints: ops read at most ONE PSUM operand and
                # gpsimd reads none. ACT stages m2/m1 to SBUF, vector does
                # the PSUM-fused adds, gpsimd the SBUF-only finals.
                ot = opool.tile([128, 2, S], mybir.dt.float16, name="ot")
                tm = tpool.tile([128, 4, S], mybir.dt.float32, name="tm")
                c2, c1, s1, s2 = (tm[:, j, :] for j in range(4))
                nc.scalar.copy(c2, ps[2][:])
                nc.scalar.copy(c1, ps[1][:])
                nc.vector.scalar_tensor_tensor(
                    s2, c2, 1.0, ps[3][:], MULT, ADD)
                nc.vector.scalar_tensor_tensor(
                    s1, c1, 1.0, ps[0][:], MULT, ADD)
                nc.gpsimd.tensor_sub(ot[:, 1, :], c1, s2)
                nc.gpsimd.tensor_add(ot[:, 0, :], s1, c2)
                # out DMA: tile index T = b*TSB + t
                T = b * TSB + t
                nc.scalar.dma_start(
                    o_d[half * 128: half * 128 + 128, n,
                        T * 2 * S: (T + 1) * 2 * S],
                    ot[:],
                )

            nextband = NBUF
            for ki, (n, b) in enumerate(order):
                for t in range(TSB):
                    for half in range(2):
                        emit_group(n, b, t, half)
                if nextband < len(order):
                    nn, nb_ = order[nextband]
                    load_band(nn, nb_)
                    nextband += 1
    nc.compile()
    return nc


def _get_nc():
    global _nc_cache
    if _nc_cache is None:
        _nc_cache = _build()
    return _nc_cache


def kernel(x, kernel):
    global LAST_RESULT
    x = np.asarray(x)
    k = np.asarray(kernel)

    wb = np.where(k >= 0, np.float32(1), np.float32(-1))  # [3,3,128,256]
    # 1D winograd weight transform along W: per dh, 4 xi planes.
    # gt[dh][xi] : [128ci, 256co]
    gt = [[wb[dh, 0],
           (wb[dh, 0] + wb[dh, 1] + wb[dh, 2]) * 0.5,
           (wb[dh, 0] - wb[dh, 1] + wb[dh, 2]) * 0.5,
           wb[dh, 2]] for dh in range(3)]
    # fp8 DR weights: slot si in {0: xi0, 1: xi3}, ktile in {dh0, dh1}
    w8 = np.zeros((CI, 2, 2, 2, 128), np.float32)
    for half in range(2):
        co = slice(half * 128, half * 128 + 128)
        for si, xi in enumerate((0, 3)):
            w8[:, half, si, 0, :] = gt[0][xi][:, co]
            w8[:, half, si, 1, :] = gt[1][xi][:, co]
    w8 = np.ascontiguousarray(w8.astype(F8))
    # fp16 weights: idx = xi*3 + dh
    w16 = np.zeros((CI, 2, 12 * 128), np.float16)
    for half in range(2):
        co = slice(half * 128, half * 128 + 128)
        for xi in range(4):
            for dh in range(3):
                idx = xi * 3 + dh
                w16[:, half, idx * 128: idx * 128 + 128] = gt[dh][xi][:, co]

    # activation transform (fp32), pad H and W by 1.
    xp = np.pad(x, ((0, 0), (1, 1), (1, 1), (0, 0)))  # [32,114,114,128]
    c_idx = np.arange(CT) * 2
    D = [xp[:, :, c_idx + j, :] for j in range(4)]  # [32,114,56,128]
    XT = [D[0] - D[2], D[1] + D[2], D[2] - D[1], D[1] - D[3]]
    # XT[xi]: [32,114,56,128] -> want [128, n, 114, xi, 56]
    xt6 = np.stack(XT, axis=2)          # [32, 114, 4, 56, 128] fp32
    xt8 = np.stack((XT[0], XT[3]), axis=2)  # [32, 114, 2, 56, 128]
    xt6 = np.ascontiguousarray(
        xt6.transpose(4, 0, 1, 2, 3).astype(np.float16))  # [128,32,114,4,56]
    xt8 = np.ascontiguousarray(
        xt8.transpose(4, 0, 1, 2, 3).astype(F8))          # [128,32,114,2,56]

    in_maps = []
    for c in range(N_CORES):
        sl = slice(c * NPC, (c + 1) * NPC)
        in_maps.append({
            "x8": xt8[:, sl].reshape(CI, NPC, HP, W8),
            "x16": xt6[:, sl].reshape(CI, NPC, HP, W6),
            "w8": w8, "w16": w16,
        })

    nc = _get_nc()
    trace = os.environ.get("BCONV_TRACE", "0") == "1"
    kwargs = {}
    if trace and os.environ.get("BCONV_TRACE_CORES", "") == "all":
        kwargs["trace_cores"] = list(range(N_CORES))
    res = run_bass_kernel_spmd(
        nc, in_maps, core_ids=list(range(N_CORES)), trace=trace, **kwargs
    )
    LAST_RESULT = res

    out = np.empty((32, H, H, CO), np.float32)
    for c in range(N_CORES):
        o = res.results[c]["out_cm"].reshape(CO, NPC, NB * TSB, 2, TROWS, CT)
        # row = 7*T + r ; col = 2*c + phase
        y = o.transpose(1, 2, 4, 5, 3, 0).reshape(NPC, H, H, CO)
        out[c * NPC: (c + 1) * NPC] = y.astype(np.float32)
    return out


# revision 11
# speedup vs baseline: 1.3444x; 1.1563x over previous
"""BinaryConnect 3x3 SAME conv (NHWC, 32x112x112x128 -> 32x112x112x256) on 8 trn2 cores.

Strategy: data-parallel (4 images/core) + 1D Winograd F(2,3) along W.
  - Host: binarize kernel (exact), 1D-Winograd-transform weights
    (coeffs in {+-1, +-1/2, +-3/2}: exact in fp16/fp8) and activations
    (4 xi planes per 2 output cols, computed in fp32, cast fp16; the two
    "edge" planes xi0=d0-d2, xi3=d1-d3 also cast to e4m3 fp8).
  - Device: m[xi] = sum_dh w~[dh,xi] . x~[row+dh, xi] accumulated in PSUM
    (3 dh matmuls per xi, 4 xi per output tile). For xi0/xi3 the dh0+dh1
    matmuls are fused into one fp8 DoubleRow matmul (k-tile steps one
    image row) -> 10 matmul slots per tile instead of direct conv's 9,
    but each covers HALF the pixels (56 col-tiles vs 112 cols): 10/18 of
    direct fp16 work. DR duty 2/10 = 20% stays under the ~22% chip
    power-throttle threshold. fp8 on the edge planes only (their error
    enters one output phase, not two): rel err ~1.76e-2 < 2e-2.
  - Vector engine reconstructs outputs from PSUM via 2 fused
    scalar_tensor_tensor ops per phase: y_even = m0+m1+m2,
    y_odd = m1-m2-m3; writes fp16 even/odd planes, host interleaves.
"""

import os

import numpy as np
import ml_dtypes

import concourse.bass as bass
import concourse.mybir as mybir
import concourse.tile as tile
from concourse import bacc
from concourse.bass_utils import run_bass_kernel_spmd

N_CORES = 8
NPC = 4            # images per core
H = 112
CT = 56            # col tiles (2 out cols each)
HP = 114           # 1 top pad + 112 rows + 1 bottom pad
W6 = 4 * CT        # fp16 x~ row width (4 xi planes)
W8 = 2 * CT        # fp8 x~ row width (xi0, xi3)
CI = 128
CO = 256
TROWS = 8          # output rows per matmul tile
S = TROWS * CT     # 448 matmul free dim (<=512 fp32 PSUM bank)
BROWS = 56         # output rows per band
NB = H // BROWS    # 2 bands per image
BIN = BROWS + 2    # x~ rows per band (1 halo row each side)
TSB = BROWS // TROWS  # 7 tiles per band
NBUF = 4           # x~ band ring depth

F8 = ml_dtypes.float8_e4m3
DR = mybir.MatmulPerfMode.DoubleRow
ADD = mybir.AluOpType.add
MULT = mybir.AluOpType.mult

_nc_cache = None
LAST_RESULT = None


def _build():
    nc = bacc.Bacc(
        "TRN2",
        target_bir_lowering=False,
        debug=False,
        num_devices=N_CORES,
    )
    x8_d = nc.dram_tensor(
        "x8", [CI, NPC, HP, W8], mybir.dt.float8e4, kind="ExternalInput"
    )
    x6_d = nc.dram_tensor(
        "x16", [CI, NPC, HP, W6], mybir.dt.float16, kind="ExternalInput"
    )
    w8_d = nc.dram_tensor(
        "w8", [CI, 2, 2, 2, 128], mybir.dt.float8e4, kind="ExternalInput"
    )
    w6_d = nc.dram_tensor(
        "w16", [CI, 2, 12 * 128], mybir.dt.float16, kind="ExternalInput"
    )
    # out: [co, n, tile(16), phase(2), S]
    o_d = nc.dram_tensor(
        "out_cm", [CO, NPC, NB * TSB * 2 * S], mybir.dt.float16,
        kind="ExternalOutput"
    )
    with tile.TileContext(nc) as tc:
        with (
            tc.tile_pool(name="x8pool", bufs=NBUF) as x8pool,
            tc.tile_pool(name="x16pool", bufs=NBUF) as x16pool,
            tc.tile_pool(name="wpool", bufs=1) as wpool,
            tc.tile_pool(name="tpool", bufs=4) as tpool,
            tc.tile_pool(name="psum", bufs=8, space=bass.MemorySpace.PSUM) as psum,
            tc.tile_pool(name="opool", bufs=8) as opool,
        ):
            # Warmup operand with no DMA dependency (memset) so PE warmup can
            # start right after the framework preamble.
            wta = wpool.tile([CI, S], mybir.dt.float16, tag="wta", name="wta")
            nc.gpsimd.memset(wta[:], 0.0)
            w8t = wpool.tile([CI, 2, 2, 2, 128], mybir.dt.float8e4, tag="w8", name="w8")
            nc.sync.dma_start(w8t[:], w8_d[:])
            w6t = wpool.tile([CI, 2, 12 * 128], mybir.dt.float16, tag="w6", name="w6")
            nc.sync.dma_start(w6t[:], w6_d[:])
            # PE warmup: throwaway matmuls to reach HAM K=8/8 before the real
            # stream begins.
            wu = psum.tile([128, S], mybir.dt.float32, name="ps")
            for _ in range(9):
                nc.tensor.matmul(
                    wu[:], wta[:, 0:128], wta[:, 0:S], start=True, stop=True
                )
            # x~ band ring: band index k = (n*NB + b), buffer k % NBUF.
            xs8, xs6 = {}, {}

            def load_band(n, b):
                k = n * NB + b
                r0 = b * BROWS
                xt8 = x8pool.tile([CI, BIN, W8], mybir.dt.float8e4, name="xb8")
                xt6 = x16pool.tile([CI, BIN, W6], mybir.dt.float16, name="xb6")
                if k == 0:
                    # first band gates early compute: split DMAs into
                    # per-tile chunks so they arrive sooner, spread queues.
                    bounds = [0, 10, 18, 26, 34, 42, 50, BIN]
                    for i in range(7):
                        ra, rb = bounds[i], bounds[i + 1]
                        q = nc.scalar if i == 0 else nc.sync
                        q.dma_start(xt8[:, ra:rb, :], x8_d[:, n, r0 + ra: r0 + rb, :])
                        q.dma_start(xt6[:, ra:rb, :], x6_d[:, n, r0 + ra: r0 + rb, :])
                else:
                    nc.sync.dma_start(xt8[:], x8_d[:, n, r0: r0 + BIN, :])
                    nc.sync.dma_start(xt6[:], x6_d[:, n, r0: r0 + BIN, :])
                xs8[n, b] = xt8
                xs6[n, b] = xt6

            # prefetch the first NBUF bands up front
            order = [(n, b) for n in range(NPC) for b in range(NB)]
            for (n, b) in order[:NBUF]:
                load_band(n, b)

            def emit_group(n, b, t, half):
                s8 = xs8[n, b]
                s6 = xs6[n, b]
                r0 = t * TROWS  # band-local x~ row base
                ps = [psum.tile([128, S], mybir.dt.float32, name="ps")
                      for _ in range(4)]
                # DR slots: xi0 (si=0), xi3 (si=1); k-tile = (dh0, dh1).
                for si in range(2):
                    nat = s8[:, r0: r0 + TROWS, si * CT: si * CT + CT]
                    pstep = nat.ap[0][0]
                    rhs = bass.AP(
                        nat.tensor, r0 * W8 + si * CT,
                        [[pstep, CI], [W8, 2], [W8, TROWS], [1, CT]],
                    )
                    xi = 0 if si == 0 else 3
                    nc.tensor.matmul(
                        ps[xi][:], w8t[:, half, si, :, :], rhs,
                        start=True, stop=False, perf_mode=DR,
                    )
                # fp16 slots, ordered so m0/m3/m2 complete early (the
                # PSUM->SBUF staging chains hang off m2 and m1).
                f16 = [(0, 2, False, True), (3, 2, False, True),
                       (2, 0, True, False), (2, 1, False, False),
                       (2, 2, False, True), (1, 0, True, False),
                       (1, 1, False, False), (1, 2, False, True)]
                for (xi, dh, st_, sp_) in f16:
                    idx = xi * 3 + dh
                    rhs = s6[:, r0 + dh: r0 + dh + TROWS,
                             xi * CT: xi * CT + CT]
                    nc.tensor.matmul(
                        ps[xi][:], w6t[:, half, idx * 128: idx * 128 + 128],
                        rhs, start=st_, stop=sp_,
                    )
                # inverse transform: even = m0+m1+m2, odd = m1-(m2+m3).
                # Engine constraints: ops read at most ONE PSUM operand and
                # gpsimd reads none. ACT stages m2/m1 to SBUF, vector does
                # the PSUM-fused adds, gpsimd the SBUF-only finals.
                ot = opool.tile([128, 2, S], mybir.dt.float16, name="ot")
                tm = tpool.tile([128, 4, S], mybir.dt.float32, name="tm")
                c2, c1, s1, s2 = (tm[:, j, :] for j in range(4))
                nc.scalar.copy(c2, ps[2][:])
                nc.scalar.copy(c1, ps[1][:])
                nc.vector.scalar_tensor_tensor(
                    s2, c2, 1.0, ps[3][:], MULT, ADD)
                nc.vector.scalar_tensor_tensor(
                    s1, c1, 1.0, ps[0][:], MULT, ADD)
                nc.gpsimd.tensor_sub(ot[:, 1, :], c1, s2)
                nc.vector.scalar_tensor_tensor(
                    ot[:, 0, :], s1, 1.0, c2, MULT, ADD)
                # out DMA: tile index T = b*TSB + t
                T = b * TSB + t
                nc.scalar.dma_start(
                    o_d[half * 128: half * 128 + 128, n,
                        T * 2 * S: (T + 1) * 2 * S],
                    ot[:],
                )

            nextband = NBUF
            for ki, (n, b) in enumerate(order):
                for t in range(TSB):
                    for half in range(2):
                        emit_group(n, b, t, half)
                if nextband < len(order):
                    nn, nb_ = order[nextband]
                    load_band(nn, nb_)
                    nextband += 1
    nc.compile()
    return nc


def _get_nc():
    global _nc_cache
    if _nc_cache is None:
        _nc_cache = _build()
    return _nc_cache


def kernel(x, kernel):
    global LAST_RESULT
    x = np.asarray(x)
    k = np.asarray(kernel)

    wb = np.where(k >= 0, np.float32(1), np.float32(-1))  # [3,3,128,256]
    # 1D winograd weight transform along W: per dh, 4 xi planes.
    # gt[dh][xi] : [128ci, 256co]
    gt = [[wb[dh, 0],
           (wb[dh, 0] + wb[dh, 1] + wb[dh, 2]) * 0.5,
           (wb[dh, 0] - wb[dh, 1] + wb[dh, 2]) * 0.5,
           wb[dh, 2]] for dh in range(3)]
    # fp8 DR weights: slot si in {0: xi0, 1: xi3}, ktile in {dh0, dh1}
    w8 = np.zeros((CI, 2, 2, 2, 128), np.float32)
    for half in range(2):
        co = slice(half * 128, half * 128 + 128)
        for si, xi in enumerate((0, 3)):
            w8[:, half, si, 0, :] = gt[0][xi][:, co]
            w8[:, half, si, 1, :] = gt[1][xi][:, co]
    w8 = np.ascontiguousarray(w8.astype(F8))
    # fp16 weights: idx = xi*3 + dh
    w16 = np.zeros((CI, 2, 12 * 128), np.float16)
    for half in range(2):
        co = slice(half * 128, half * 128 + 128)
        for xi in range(4):
            for dh in range(3):
                idx = xi * 3 + dh
                w16[:, half, idx * 128: idx * 128 + 128] = gt[dh][xi][:, co]

    # activation transform (fp32), pad H and W by 1.
    xp = np.pad(x, ((0, 0), (1, 1), (1, 1), (0, 0)))  # [32,114,114,128]
    c_idx = np.arange(CT) * 2
    D = [xp[:, :, c_idx + j, :] for j in range(4)]  # [32,114,56,128]
    XT = [D[0] - D[2], D[1] + D[2], D[2] - D[1], D[1] - D[3]]
    # XT[xi]: [32,114,56,128] -> want [128, n, 114, xi, 56]
    xt6 = np.stack(XT, axis=2)          # [32, 114, 4, 56, 128] fp32
    xt8 = np.stack((XT[0], XT[3]), axis=2)  # [32, 114, 2, 56, 128]
    xt6 = np.ascontiguousarray(
        xt6.transpose(4, 0, 1, 2, 3).astype(np.float16))  # [128,32,114,4,56]
    xt8 = np.ascontiguousarray(
        xt8.transpose(4, 0, 1, 2, 3).astype(F8))          # [128,32,114,2,56]

    in_maps = []
    for c in range(N_CORES):
        sl = slice(c * NPC, (c + 1) * NPC)
        in_maps.append({
            "x8": xt8[:, sl].reshape(CI, NPC, HP, W8),
            "x16": xt6[:, sl].reshape(CI, NPC, HP, W6),
            "w8": w8, "w16": w16,
        })

    nc = _get_nc()
    trace = os.environ.get("BCONV_TRACE", "0") == "1"
    kwargs = {}
    if trace and os.environ.get("BCONV_TRACE_CORES", "") == "all":
        kwargs["trace_cores"] = list(range(N_CORES))
    res = run_bass_kernel_spmd(
        nc, in_maps, core_ids=list(range(N_CORES)), trace=trace, **kwargs
    )
    LAST_RESULT = res

    out = np.empty((32, H, H, CO), np.float32)
    for c in range(N_CORES):
        o = res.results[c]["out_cm"].reshape(CO, NPC, NB * TSB, 2, TROWS, CT)
        # row = 7*T + r ; col = 2*c + phase
        y = o.transpose(1, 2, 4, 5, 3, 0).reshape(NPC, H, H, CO)
        out[c * NPC: (c + 1) * NPC] = y.astype(np.float32)
    return out
